# revision 1
# baseline (speedup 1.0000x reference)
"""LoraLinear (int8-dequant matmul + low-rank LoRA) on 8 trn2 NeuronCores.

out[b,s,o] = sum_i x[b,s,i]*q[o,i]*scale[o] + 2.0 * sum_r (sum_i x[b,s,i]*A[r,i]) * B[o,r]

Strategy: data-parallel over the 8192 flattened tokens (1024/core, no
collectives). Host folds scale into the weight, casts operands to bf16
(int8 codes are exact in bf16), and pre-transposes so every DMA is
contiguous. On device each core does a plain bf16 matmul with fp32 PSUM
accumulation; the LoRA term is folded into the same PSUM accumulation
group as one extra K=64 matmul per output tile.
"""

import numpy as np
import ml_dtypes

BF16 = ml_dtypes.bfloat16

B, S, DIN, DOUT, R = 4, 2048, 4096, 4096, 64
N_CORES = 8
TOK = B * S  # 8192
T = TOK // N_CORES  # 1024 tokens per core
P = 128
IC = DIN // P  # 32 contraction chunks
O_TILE = 512
N_OT = DOUT // O_TILE  # 8
N_TT = T // P  # 8
SCALING = 2.0

_CACHE = {}


def build_nc():
    import concourse.mybir as mybir
    import concourse.tile as tile
    from concourse import bacc

    dt = mybir.dt
    nc = bacc.Bacc("TRN2", target_bir_lowering=False, debug=False,
                   num_devices=N_CORES)

    xT_d = nc.dram_tensor("xT", [P, IC, T], dt.bfloat16, kind="ExternalInput").ap()
    wT_d = nc.dram_tensor("wT", [N_OT, P, IC, O_TILE], dt.bfloat16, kind="ExternalInput").ap()
    aT_d = nc.dram_tensor("aT", [P, IC, R], dt.bfloat16, kind="ExternalInput").ap()
    b2T_d = nc.dram_tensor("b2T", [R, DOUT], dt.bfloat16, kind="ExternalInput").ap()
    out_d = nc.dram_tensor("out", [N_OT, N_TT, P, O_TILE], dt.float32, kind="ExternalOutput").ap()

    XCH = 1   # ic per xT tile chunk -> 32 chunks
    WCH = 4   # ic per w tile chunk  -> 8 chunks

    with tile.TileContext(nc) as tc:
        with (
            tc.tile_pool(name="xpool", bufs=1) as xpool,
            tc.tile_pool(name="wpool", bufs=2) as wpool,
            tc.tile_pool(name="cpool", bufs=1) as cpool,
            tc.tile_pool(name="opool", bufs=4) as opool,
            tc.tile_pool(name="psmain", bufs=6, space="PSUM") as psmain,
            tc.tile_pool(name="psxa", bufs=2, space="PSUM") as psxa,
        ):
            # xT and aT split into independently-DMA'd tiles so PE can stream
            # behind the loads (Tile deps are tile-granular).
            ACH = 8
            ats = [cpool.tile([P, ACH, R], dt.bfloat16, tag=f"at{i}", name=f"at{i}")
                   for i in range(IC // ACH)]
            xts = [xpool.tile([P, XCH, T], dt.bfloat16, tag=f"xt{i}", name=f"xt{i}")
                   for i in range(IC // XCH)]

            def x_sl(ic, lo, hi):
                return xts[ic // XCH][:, ic % XCH, lo:hi]

            def a_sl(ic):
                return ats[ic // ACH][:, ic % ACH, :]

            def w_tiles(ot):
                ws = [wpool.tile([P, WCH, O_TILE], dt.bfloat16, tag=f"w{q}", name=f"w_{q}")
                      for q in range(IC // WCH)]
                for q, w in enumerate(ws):
                    nc.sync.dma_start(w[:], wT_d[ot, :, WCH * q:WCH * (q + 1), :])
                return ws

            def w_sl(ws, ic):
                return ws[ic // WCH][:, ic % WCH, :]

            # interleaved DMA emission: x chunks and first w chunks stream
            # together so the ic-outer phase below is PE-bound from the start
            b2T = cpool.tile([R, DOUT], dt.bfloat16)
            w0 = [wpool.tile([P, WCH, O_TILE], dt.bfloat16, tag=f"w{q}", name=f"w0_{q}")
                  for q in range(IC // WCH)]
            nxt = len(xts)
            done_w = 0
            nc.sync.dma_start(ats[0][:], aT_d[:, 0:ACH, :])
            for j in range(nxt):
                nc.sync.dma_start(xts[j][:], xT_d[:, XCH * j:XCH * (j + 1), :])
                if j == 4:
                    nc.sync.dma_start(ats[1][:], aT_d[:, ACH:2 * ACH, :])
                elif j == 10:
                    nc.sync.dma_start(ats[2][:], aT_d[:, 2 * ACH:3 * ACH, :])
                elif j == 16:
                    nc.sync.dma_start(ats[3][:], aT_d[:, 3 * ACH:4 * ACH, :])
                if j % 4 == 1 and done_w < IC // WCH:  # w chunk after every 4th x chunk
                    nc.sync.dma_start(w0[done_w][:],
                                      wT_d[0, :, WCH * done_w:WCH * (done_w + 1), :])
                    done_w += 1
            nc.sync.dma_start(b2T[:], b2T_d[:])

            xaT = cpool.tile([R, T], dt.bfloat16)
            NB = T // O_TILE  # xa psum blocks (2)

            def lora_and_evict(ps, ot, tt):
                nc.tensor.matmul(
                    ps[:], xaT[:, tt * P:(tt + 1) * P],
                    b2T[:, ot * O_TILE:(ot + 1) * O_TILE],
                    start=False, stop=True,
                )
                st = opool.tile([P, O_TILE], dt.float32)
                # split the eviction across DVE and ACT, each half pipelined
                # straight into its own store DMA, so the post-matmul chain is
                # max(copy)+half-DMA instead of copy+full-DMA
                h = O_TILE // 2
                nc.vector.tensor_copy(out=st[:, :h], in_=ps[:, :h])
                nc.sync.dma_start(out_d[ot, tt, :, 0:h], st[:, :h])
                nc.scalar.copy(st[:, h:], ps[:, h:])
                nc.sync.dma_start(out_d[ot, tt, :, h:O_TILE], st[:, h:])

            # ---- phase 1 (ot=0): ic-outer, xa + 4 token groups interleaved
            ps_xa = [psxa.tile([R, O_TILE], dt.float32, tag="psxa", name=f"psxa{b}") for b in range(NB)]
            NPG = 6
            ps_g = [psmain.tile([P, O_TILE], dt.float32, tag="ps", name=f"psg{g}") for g in range(NPG)]
            for ic in range(IC):
                for tb in range(NB):
                    nc.tensor.matmul(
                        ps_xa[tb][:], a_sl(ic),
                        x_sl(ic, tb * O_TILE, (tb + 1) * O_TILE),
                        start=(ic == 0), stop=(ic == IC - 1),
                    )
                for tt in range(NPG):
                    nc.tensor.matmul(
                        ps_g[tt][:], x_sl(ic, tt * P, (tt + 1) * P), w_sl(w0, ic),
                        start=(ic == 0), stop=False,
                    )
            for tb in range(NB):
                nc.any.tensor_copy(out=xaT[:, tb * O_TILE:(tb + 1) * O_TILE],
                                   in_=ps_xa[tb][:])
            for tt in range(NPG):
                lora_and_evict(ps_g[tt], 0, tt)
            # ot=0 remaining token groups (everything resident)
            for tt in range(NPG, N_TT):
                ps = psmain.tile([P, O_TILE], dt.float32, tag="ps", name="ps")
                for ic in range(IC):
                    nc.tensor.matmul(
                        ps[:], x_sl(ic, tt * P, (tt + 1) * P), w_sl(w0, ic),
                        start=(ic == 0), stop=False,
                    )
                lora_and_evict(ps, 0, tt)

            # ---- steady state: ot = 1..7
            for ot in range(1, N_OT):
                ws = w_tiles(ot)
                for tt in range(N_TT):
                    ps = psmain.tile([P, O_TILE], dt.float32, tag="ps", name="ps")
                    for ic in range(IC):
                        nc.tensor.matmul(
                            ps[:], x_sl(ic, tt * P, (tt + 1) * P), w_sl(ws, ic),
                            start=(ic == 0), stop=False,
                        )
                    lora_and_evict(ps, ot, tt)

    nc.compile()
    return nc


def _prep_inputs(x, qweight, scale, lora_A, lora_B):
    x_flat = np.ascontiguousarray(x.reshape(TOK, DIN))
    # xT per core: [P, IC, T], row i = ic*P + p
    xT_all = x_flat.T.astype(BF16)  # [DIN, TOK]
    per_core_xT = []
    for c in range(N_CORES):
        xs = xT_all[:, c * T:(c + 1) * T]
        per_core_xT.append(np.ascontiguousarray(
            xs.reshape(IC, P, T).transpose(1, 0, 2)))
    # weight with scale folded, transposed: wT[i, o]
    w = qweight.astype(np.float32) * scale.astype(np.float32)  # [DOUT, DIN]
    wT = w.T.astype(BF16)  # [DIN, DOUT]
    wT_t = np.ascontiguousarray(
        wT.reshape(IC, P, N_OT, O_TILE).transpose(2, 1, 0, 3))  # [N_OT, P, IC, O_TILE]
    aT = np.ascontiguousarray(
        lora_A.T.astype(BF16).reshape(IC, P, R).transpose(1, 0, 2))  # [P, IC, R]
    b2T = np.ascontiguousarray((SCALING * lora_B).T.astype(BF16))  # [R, DOUT]
    return per_core_xT, wT_t, aT, b2T


def run(x, qweight, scale, lora_A, lora_B, trace=False):
    from concourse.bass_utils import run_bass_kernel_spmd

    if "nc" not in _CACHE:
        _CACHE["nc"] = build_nc()
    nc = _CACHE["nc"]

    per_core_xT, wT_t, aT, b2T = _prep_inputs(x, qweight, scale, lora_A, lora_B)
    in_maps = [
        {"xT": per_core_xT[c], "wT": wT_t, "aT": aT, "b2T": b2T}
        for c in range(N_CORES)
    ]
    res = run_bass_kernel_spmd(nc, in_maps, core_ids=list(range(N_CORES)),
                               trace=trace)
    outs = []
    for c in range(N_CORES):
        o = res.results[c]["out"]  # [N_OT, N_TT, P, O_TILE]
        outs.append(o.transpose(1, 2, 0, 3).reshape(T, DOUT))
    full = np.concatenate(outs, axis=0).reshape(B, S, DOUT).astype(np.float32)
    return full, res


def kernel(x, qweight, scale, lora_A, lora_B):
    full, _ = run(x, qweight, scale, lora_A, lora_B)
    return full



# revision 2
# speedup vs baseline: 1.3520x; 1.3520x over previous
"""LoraLinear (int8-dequant matmul + low-rank LoRA) on 8 trn2 NeuronCores.

out[b,s,o] = sum_i x[b,s,i]*q[o,i]*scale[o] + 2.0 * sum_r (sum_i x[b,s,i]*A[r,i]) * B[o,r]

Strategy: data-parallel over the 8192 flattened tokens (1024/core, no
collectives). The LoRA update is dense-folded on the host into the
effective weight W_eff = q*scale + 2*B@A, so the device does a single
GEMM. W_eff and x are each split into two exact-ish fp8 e4m3 planes
(hi = rne(v), lo = rne(v - hi)); three of the four plane cross-products
are computed (hi*hi + lo*hi + hi*lo), leaving only the lo*lo term as
error (~1.3e-3 rel). All matmuls run in DoubleRow perf mode (fp8,
K=256 per instruction, 0.5 cycles per output element = 4x bf16 MAC
throughput), accumulating the three passes in fp32 PSUM before one
eviction per output tile.
"""

import numpy as np
import ml_dtypes

E4 = ml_dtypes.float8_e4m3

B, S, DIN, DOUT, R = 4, 2048, 4096, 4096, 64
N_CORES = 8
TOK = B * S  # 8192
T = TOK // N_CORES  # 1024 tokens per core
P = 128
KG = DIN // 256  # 16 K-groups, each 2x128 contraction per DoubleRow matmul
O_TILE = 512
N_OT = DOUT // O_TILE  # 8
N_TT = T // P  # 8
WCH = 4  # kg per W DMA chunk
SCALING = 2.0

_CACHE = {}


def build_nc():
    import concourse.mybir as mybir
    import concourse.tile as tile
    from concourse import bacc

    dt = mybir.dt
    DR = mybir.MatmulPerfMode.DoubleRow
    nc = bacc.Bacc("TRN2", target_bir_lowering=False, debug=False,
                   num_devices=N_CORES)

    xq_d = nc.dram_tensor("xq", [P, KG, 2, T], dt.float8e4, kind="ExternalInput").ap()
    xr_d = nc.dram_tensor("xr", [P, KG, 2, T], dt.float8e4, kind="ExternalInput").ap()
    wq_d = nc.dram_tensor("wq", [N_OT, P, KG, 2, O_TILE], dt.float8e4, kind="ExternalInput").ap()
    wr_d = nc.dram_tensor("wr", [N_OT, P, KG, 2, O_TILE], dt.float8e4, kind="ExternalInput").ap()
    out_d = nc.dram_tensor("out", [N_OT, N_TT, P, O_TILE], dt.float32, kind="ExternalOutput").ap()

    NCH = KG // WCH  # 4 W chunks per (plane, ot)

    with tile.TileContext(nc) as tc:
        with (
            tc.tile_pool(name="xpool", bufs=1) as xpool,
            tc.tile_pool(name="wpool", bufs=2) as wpool,
            tc.tile_pool(name="opool", bufs=4) as opool,
            tc.tile_pool(name="pspool", bufs=8, space="PSUM") as pspool,
        ):
            xq_t = [xpool.tile([P, 2, T], dt.float8e4, tag=f"xq{k}", name=f"xq{k}")
                    for k in range(KG)]
            xr_t = [xpool.tile([P, 2, T], dt.float8e4, tag=f"xr{k}", name=f"xr{k}")
                    for k in range(KG)]

            def alloc_w(ot):
                wq = [wpool.tile([P, WCH, 2, O_TILE], dt.float8e4, tag=f"wq{c}",
                                 name=f"wq{ot}_{c}") for c in range(NCH)]
                wr = [wpool.tile([P, WCH, 2, O_TILE], dt.float8e4, tag=f"wr{c}",
                                 name=f"wr{ot}_{c}") for c in range(NCH)]
                return wq, wr

            def dma_w_chunk(ws, w_d, ot, c):
                nc.sync.dma_start(ws[c][:], w_d[ot, :, WCH * c:WCH * (c + 1), :, :])

            def w_sl(ws, kg):
                return ws[kg // WCH][:, kg % WCH, :, :]

            # ---- prologue DMA: W chunk (both planes) ahead of its 4 x-pairs so
            # ot=0 compute streams kg-by-kg behind the loads
            w0q, w0r = alloc_w(0)
            for k in range(KG):
                if k % WCH == 0:
                    dma_w_chunk(w0q, wq_d, 0, k // WCH)
                    dma_w_chunk(w0r, wr_d, 0, k // WCH)
                nc.sync.dma_start(xq_t[k][:], xq_d[:, k, :, :])
                nc.sync.dma_start(xr_t[k][:], xr_d[:, k, :, :])

            def evict(ps, ot, tt):
                st = opool.tile([P, O_TILE], dt.float32)
                h = O_TILE // 2
                nc.vector.tensor_copy(out=st[:, :h], in_=ps[:, :h])
                nc.sync.dma_start(out_d[ot, tt, :, 0:h], st[:, :h])
                nc.scalar.copy(st[:, h:], ps[:, h:])
                nc.sync.dma_start(out_d[ot, tt, :, h:O_TILE], st[:, h:])

            # ---- ot = 0: fully kg-outer (all 3 plane-passes per kg) so PE
            # starts as soon as the first chunk lands
            ps0 = [pspool.tile([P, O_TILE], dt.float32, tag="ps", name=f"ps0_{t}")
                   for t in range(N_TT)]
            for k in range(KG):
                for xp, wp, first, last in (
                    (xq_t, w0q, True, False),
                    (xq_t, w0r, False, False),
                    (xr_t, w0q, False, True),
                ):
                    for tt in range(N_TT):
                        nc.tensor.matmul(
                            ps0[tt][:], xp[k][:, :, tt * P:(tt + 1) * P],
                            w_sl(wp, k),
                            start=(first and k == 0), stop=(last and k == KG - 1),
                            perf_mode=DR,
                        )
            for tt in range(N_TT):
                evict(ps0[tt], 0, tt)

            # ---- ot = 1..7: prefetched weights; passes (q,q)+(r,q) kg-outer,
            # final (q,r) pass tt-outer so evictions spread out
            wq_c, wr_c = alloc_w(1)
            for c in range(NCH):
                dma_w_chunk(wq_c, wq_d, 1, c)
                dma_w_chunk(wr_c, wr_d, 1, c)
            for ot in range(1, N_OT):
                wq, wr = wq_c, wr_c
                if ot + 1 < N_OT:
                    wq_c, wr_c = alloc_w(ot + 1)
                    for c in range(NCH):
                        dma_w_chunk(wq_c, wq_d, ot + 1, c)
                        dma_w_chunk(wr_c, wr_d, ot + 1, c)
                ps = [pspool.tile([P, O_TILE], dt.float32, tag="ps", name=f"ps{ot}_{t}")
                      for t in range(N_TT)]
                for k in range(KG):
                    for xp, wp, first in ((xq_t, wq, True), (xq_t, wr, False)):
                        for tt in range(N_TT):
                            nc.tensor.matmul(
                                ps[tt][:], xp[k][:, :, tt * P:(tt + 1) * P],
                                w_sl(wp, k),
                                start=(first and k == 0), stop=False,
                                perf_mode=DR,
                            )
                for tt in range(N_TT):
                    for k in range(KG):
                        nc.tensor.matmul(
                            ps[tt][:], xr_t[k][:, :, tt * P:(tt + 1) * P],
                            w_sl(wq, k),
                            start=False, stop=(k == KG - 1),
                            perf_mode=DR,
                        )
                    evict(ps[tt], ot, tt)

    nc.compile()
    return nc


def _split_planes(v):
    hi = v.astype(E4)
    lo = (v - hi.astype(np.float32)).astype(E4)
    return hi, lo


def _prep_inputs(x, qweight, scale, lora_A, lora_B):
    # effective dense weight with the LoRA update folded in
    w = qweight.astype(np.float32) * scale.astype(np.float32)
    w += SCALING * (lora_B.astype(np.float32) @ lora_A.astype(np.float32))
    wq, wr = _split_planes(w)

    def w_layout(p):
        # [DOUT, DIN] -> K-major rhs layout [N_OT, P, KG, 2, O_TILE],
        # K = kg*256 + sub*128 + p
        t = p.T.reshape(KG, 2, P, N_OT, O_TILE)
        return np.ascontiguousarray(t.transpose(3, 2, 0, 1, 4))

    xf = np.ascontiguousarray(x.reshape(TOK, DIN))
    xhi, xlo = _split_planes(xf)

    def x_layout(p, c):
        # core slice [T, DIN] -> lhsT layout [P, KG, 2, T]
        t = p[c * T:(c + 1) * T].T.reshape(KG, 2, P, T)
        return np.ascontiguousarray(t.transpose(2, 0, 1, 3))

    wq_l, wr_l = w_layout(wq), w_layout(wr)
    per_core = [
        {"xq": x_layout(xhi, c), "xr": x_layout(xlo, c), "wq": wq_l, "wr": wr_l}
        for c in range(N_CORES)
    ]
    return per_core


def run(x, qweight, scale, lora_A, lora_B, trace=False):
    from concourse.bass_utils import run_bass_kernel_spmd

    if "nc" not in _CACHE:
        _CACHE["nc"] = build_nc()
    nc = _CACHE["nc"]

    in_maps = _prep_inputs(x, qweight, scale, lora_A, lora_B)
    res = run_bass_kernel_spmd(nc, in_maps, core_ids=list(range(N_CORES)),
                               trace=trace)
    outs = []
    for c in range(N_CORES):
        o = res.results[c]["out"]  # [N_OT, N_TT, P, O_TILE]
        outs.append(o.transpose(1, 2, 0, 3).reshape(T, DOUT))
    full = np.concatenate(outs, axis=0).reshape(B, S, DOUT).astype(np.float32)
    return full, res


def kernel(x, qweight, scale, lora_A, lora_B):
    full, _ = run(x, qweight, scale, lora_A, lora_B)
    return full


# revision 4
# speedup vs baseline: 1.3821x; 1.0223x over previous
"""LoraLinear (int8-dequant matmul + low-rank LoRA) on 8 trn2 NeuronCores.

out[b,s,o] = sum_i x[b,s,i]*q[o,i]*scale[o] + 2.0 * sum_r (sum_i x[b,s,i]*A[r,i]) * B[o,r]

Strategy: data-parallel over the 8192 flattened tokens (1024/core, no
collectives). The LoRA update is dense-folded on the host into the
effective weight W_eff = q*scale + 2*B@A, so the device does a single
GEMM. W_eff and x are each split into two exact-ish fp8 e4m3 planes
(hi = rne(v), lo = rne(v - hi)); three of the four plane cross-products
are computed (hi*hi + lo*hi + hi*lo), leaving only the lo*lo term as
error (~1.3e-3 rel). All matmuls run in DoubleRow perf mode (fp8,
K=256 per instruction, 0.5 cycles per output element = 4x bf16 MAC
throughput), accumulating the three passes in fp32 PSUM before one
eviction per output tile.

Pipeline details: 8 persistent PSUM tiles (one per token tile) give
precise per-bank WAR deps across output-column tiles; a dozen dummy
matmuls on a zeroed SBUF tile warm the PE p-state ramp during the
prologue DMA; ot=0 runs all three passes kg-streamed behind the loads
with a tt-outer tail so evictions spread; later ots prefetch weights
one tile ahead.
"""

import numpy as np
import ml_dtypes

E4 = ml_dtypes.float8_e4m3

B, S, DIN, DOUT, R = 4, 2048, 4096, 4096, 64
N_CORES = 8
TOK = B * S  # 8192
T = TOK // N_CORES  # 1024 tokens per core
P = 128
KG = DIN // 256  # 16 K-groups, each 2x128 contraction per DoubleRow matmul
O_TILE = 512
N_OT = DOUT // O_TILE  # 8
N_TT = T // P  # 8
WCH = 2  # kg per W DMA chunk
NCH = KG // WCH  # 8 W chunks per (plane, ot)
SCALING = 2.0
N_WARM = 12  # PE p-state warmup matmuls
KG_TAIL = 2  # kg processed tt-outer at the end of ot=0

_CACHE = {}


def build_nc():
    import concourse.mybir as mybir
    import concourse.tile as tile
    from concourse import bacc

    dt = mybir.dt
    DR = mybir.MatmulPerfMode.DoubleRow
    nc = bacc.Bacc("TRN2", target_bir_lowering=False, debug=False,
                   num_devices=N_CORES)

    xq_d = nc.dram_tensor("xq", [P, KG, 2, T], dt.float8e4, kind="ExternalInput").ap()
    xr_d = nc.dram_tensor("xr", [P, KG, 2, T], dt.float8e4, kind="ExternalInput").ap()
    wq_d = nc.dram_tensor("wq", [N_OT, P, KG, 2, O_TILE], dt.float8e4, kind="ExternalInput").ap()
    wr_d = nc.dram_tensor("wr", [N_OT, P, KG, 2, O_TILE], dt.float8e4, kind="ExternalInput").ap()
    out_d = nc.dram_tensor("out", [N_OT, N_TT, P, O_TILE], dt.float32, kind="ExternalOutput").ap()

    with tile.TileContext(nc) as tc:
        with (
            tc.tile_pool(name="xpool", bufs=1) as xpool,
            tc.tile_pool(name="wpool", bufs=2) as wpool,
            tc.tile_pool(name="opool", bufs=4) as opool,
            tc.tile_pool(name="pspool", bufs=1, space="PSUM") as pspool,
        ):
            # persistent PSUM tiles, one per token tile; reused every ot so
            # WAR deps are per-bank (matmul waits only on its own bank's
            # eviction, not a pool-rotation barrier)
            ps = [pspool.tile([P, O_TILE], dt.float32, tag=f"ps{t}", name=f"ps{t}")
                  for t in range(N_TT)]

            # warmup: PE ramps to full p-state during the prologue DMAs
            z = xpool.tile([P, 2, O_TILE], dt.float8e4, tag="z", name="z")
            nc.vector.memset(z[:], 0)
            for i in range(N_WARM):
                nc.tensor.matmul(ps[0][:], z[:, :, :P], z[:],
                                 start=True, stop=True, perf_mode=DR)

            xq_t = [xpool.tile([P, 2, T], dt.float8e4, tag=f"xq{k}", name=f"xq{k}")
                    for k in range(KG)]
            xr_t = [xpool.tile([P, 2, T], dt.float8e4, tag=f"xr{k}", name=f"xr{k}")
                    for k in range(KG)]

            def alloc_w(ot):
                wq = [wpool.tile([P, WCH, 2, O_TILE], dt.float8e4, tag=f"wq{c}",
                                 name=f"wq{ot}_{c}") for c in range(NCH)]
                wr = [wpool.tile([P, WCH, 2, O_TILE], dt.float8e4, tag=f"wr{c}",
                                 name=f"wr{ot}_{c}") for c in range(NCH)]
                return wq, wr

            def dma_w_chunk(ws, w_d, ot, c):
                nc.sync.dma_start(ws[c][:], w_d[ot, :, WCH * c:WCH * (c + 1), :, :])

            def w_sl(ws, kg):
                return ws[kg // WCH][:, kg % WCH, :, :]

            # prologue DMA: interleaved in exactly the order ot=0 consumes
            w0q, w0r = alloc_w(0)
            for c in range(NCH):
                dma_w_chunk(w0q, wq_d, 0, c)
                nc.sync.dma_start(xq_t[2 * c][:], xq_d[:, 2 * c, :, :])
                dma_w_chunk(w0r, wr_d, 0, c)
                nc.sync.dma_start(xr_t[2 * c][:], xr_d[:, 2 * c, :, :])
                nc.sync.dma_start(xq_t[2 * c + 1][:], xq_d[:, 2 * c + 1, :, :])
                nc.sync.dma_start(xr_t[2 * c + 1][:], xr_d[:, 2 * c + 1, :, :])

            def evict(tt, ot, last=False):
                st = opool.tile([P, O_TILE], dt.float32)
                h = 3 * O_TILE // 4 if last else O_TILE // 2
                nc.vector.tensor_copy(out=st[:, :h], in_=ps[tt][:, :h])
                nc.sync.dma_start(out_d[ot, tt, :, 0:h], st[:, :h])
                nc.scalar.copy(st[:, h:], ps[tt][:, h:])
                nc.sync.dma_start(out_d[ot, tt, :, h:O_TILE], st[:, h:])

            PASSES = ((xq_t, "q"), (xq_t, "r"), (xr_t, "q"))

            def mm(tt, kg, xp, wsel, wq, wr, start, stop):
                nc.tensor.matmul(
                    ps[tt][:], xp[kg][:, :, tt * P:(tt + 1) * P],
                    w_sl(wq if wsel == "q" else wr, kg),
                    start=start, stop=stop, perf_mode=DR,
                )

            # ---- ot = 0: kg-streamed, all 3 passes per kg; last KG_TAIL
            # kgs go tt-outer so the 8 evictions spread out
            for k in range(KG - KG_TAIL):
                for pi, (xp, wsel) in enumerate(PASSES):
                    for tt in range(N_TT):
                        mm(tt, k, xp, wsel, w0q, w0r,
                           start=(pi == 0 and k == 0), stop=False)
            for tt in range(N_TT):
                for k in range(KG - KG_TAIL, KG):
                    for pi, (xp, wsel) in enumerate(PASSES):
                        mm(tt, k, xp, wsel, w0q, w0r, start=False,
                           stop=(pi == len(PASSES) - 1 and k == KG - 1))
                evict(tt, 0)

            # ---- ot = 1..7: weights prefetched an ot ahead; (q,q)+(r,q)
            # kg-outer, final (q,r) pass tt-outer with spread evictions
            wq_c, wr_c = alloc_w(1)
            for c in range(NCH):
                dma_w_chunk(wq_c, wq_d, 1, c)
                dma_w_chunk(wr_c, wr_d, 1, c)
            for ot in range(1, N_OT):
                wq, wr = wq_c, wr_c
                if ot + 1 < N_OT:
                    wq_c, wr_c = alloc_w(ot + 1)
                    for c in range(NCH):
                        dma_w_chunk(wq_c, wq_d, ot + 1, c)
                        dma_w_chunk(wr_c, wr_d, ot + 1, c)
                for k in range(KG):
                    for xp, wsel, first in ((xq_t, "q", True), (xq_t, "r", False)):
                        for tt in range(N_TT):
                            mm(tt, k, xp, wsel, wq, wr,
                               start=(first and k == 0), stop=False)
                for tt in range(N_TT):
                    for k in range(KG):
                        mm(tt, k, xr_t, "q", wq, wr, start=False,
                           stop=(k == KG - 1))
                    evict(tt, ot, last=(ot == N_OT - 1 and tt == N_TT - 1))

    nc.compile()
    return nc


def _split_planes(v):
    hi = v.astype(E4)
    lo = (v - hi.astype(np.float32)).astype(E4)
    return hi, lo


def _prep_inputs(x, qweight, scale, lora_A, lora_B):
    # effective dense weight with the LoRA update folded in
    w = qweight.astype(np.float32) * scale.astype(np.float32)
    w += SCALING * (lora_B.astype(np.float32) @ lora_A.astype(np.float32))
    wq, wr = _split_planes(w)

    def w_layout(p):
        # [DOUT, DIN] -> K-major rhs layout [N_OT, P, KG, 2, O_TILE],
        # K = kg*256 + sub*128 + p
        t = p.T.reshape(KG, 2, P, N_OT, O_TILE)
        return np.ascontiguousarray(t.transpose(3, 2, 0, 1, 4))

    xf = np.ascontiguousarray(x.reshape(TOK, DIN))
    xhi, xlo = _split_planes(xf)

    def x_layout(p, c):
        # core slice [T, DIN] -> lhsT layout [P, KG, 2, T]
        t = p[c * T:(c + 1) * T].T.reshape(KG, 2, P, T)
        return np.ascontiguousarray(t.transpose(2, 0, 1, 3))

    wq_l, wr_l = w_layout(wq), w_layout(wr)
    per_core = [
        {"xq": x_layout(xhi, c), "xr": x_layout(xlo, c), "wq": wq_l, "wr": wr_l}
        for c in range(N_CORES)
    ]
    return per_core


def run(x, qweight, scale, lora_A, lora_B, trace=False):
    from concourse.bass_utils import run_bass_kernel_spmd

    if "nc" not in _CACHE:
        _CACHE["nc"] = build_nc()
    nc = _CACHE["nc"]

    in_maps = _prep_inputs(x, qweight, scale, lora_A, lora_B)
    res = run_bass_kernel_spmd(nc, in_maps, core_ids=list(range(N_CORES)),
                               trace=trace)
    outs = []
    for c in range(N_CORES):
        o = res.results[c]["out"]  # [N_OT, N_TT, P, O_TILE]
        outs.append(o.transpose(1, 2, 0, 3).reshape(T, DOUT))
    full = np.concatenate(outs, axis=0).reshape(B, S, DOUT).astype(np.float32)
    return full, res


def kernel(x, qweight, scale, lora_A, lora_B):
    full, _ = run(x, qweight, scale, lora_A, lora_B)
    return full


# revision 8
# speedup vs baseline: 1.3962x; 1.0102x over previous
"""LoraLinear (int8-dequant matmul + low-rank LoRA) on 8 trn2 NeuronCores.

out[b,s,o] = sum_i x[b,s,i]*q[o,i]*scale[o] + 2.0 * sum_r (sum_i x[b,s,i]*A[r,i]) * B[o,r]

Strategy: data-parallel over the 8192 flattened tokens (1024/core, no
collectives). The LoRA update is dense-folded on the host into the
effective weight W_eff = q*scale + 2*B@A, so the device does a single
GEMM. W_eff and x are each split into two exact-ish fp8 e4m3 planes
(hi = rne(v), lo = rne(v - hi)); three of the four plane cross-products
are computed (hi*hi + lo*hi + hi*lo), leaving only the lo*lo term as
error (~1.3e-3 rel). All matmuls run in DoubleRow perf mode (fp8,
K=256 per instruction, 0.5 cycles per output element = 4x bf16 MAC
throughput), accumulating the three passes in fp32 PSUM before one
eviction per output tile.

Pipeline details: 8 persistent PSUM tiles (one per token tile) give
precise per-bank WAR deps across output-column tiles; a dozen dummy
matmuls on a zeroed SBUF tile warm the PE p-state ramp during the
prologue DMA; ot=0 runs all three passes kg-streamed behind the loads
with a tt-outer tail so evictions spread; later ots prefetch weights
one tile ahead.
"""

import numpy as np
import ml_dtypes

E4 = ml_dtypes.float8_e4m3

B, S, DIN, DOUT, R = 4, 2048, 4096, 4096, 64
N_CORES = 8
TOK = B * S  # 8192
T = TOK // N_CORES  # 1024 tokens per core
P = 128
KG = DIN // 256  # 16 K-groups, each 2x128 contraction per DoubleRow matmul
O_TILE = 512
N_OT = DOUT // O_TILE  # 8
N_TT = T // P  # 8
WCH = 2  # kg per W DMA chunk
NCH = KG // WCH  # 8 W chunks per (plane, ot)
SCALING = 2.0
N_WARM = 72  # PE p-state warmup matmuls (128-wide, ~3us at ramp speeds)
KG_TAIL = 2  # kg processed tt-outer at the end of ot=0

_CACHE = {}


def build_nc():
    import concourse.mybir as mybir
    import concourse.tile as tile
    from concourse import bacc

    dt = mybir.dt
    DR = mybir.MatmulPerfMode.DoubleRow
    nc = bacc.Bacc("TRN2", target_bir_lowering=False, debug=False,
                   num_devices=N_CORES)

    xq_d = nc.dram_tensor("xq", [P, KG, 2, T], dt.float8e4, kind="ExternalInput").ap()
    xr_d = nc.dram_tensor("xr", [P, KG, 2, T], dt.float8e4, kind="ExternalInput").ap()
    wq_d = nc.dram_tensor("wq", [N_OT, P, KG, 2, O_TILE], dt.float8e4, kind="ExternalInput").ap()
    wr_d = nc.dram_tensor("wr", [N_OT, P, KG, 2, O_TILE], dt.float8e4, kind="ExternalInput").ap()
    out_d = nc.dram_tensor("out", [N_OT, N_TT, P, O_TILE], dt.float32, kind="ExternalOutput").ap()

    with tile.TileContext(nc) as tc:
        with (
            tc.tile_pool(name="xpool", bufs=1) as xpool,
            tc.tile_pool(name="wpool", bufs=2) as wpool,
            tc.tile_pool(name="opool", bufs=4) as opool,
            tc.tile_pool(name="pspool", bufs=1, space="PSUM") as pspool,
        ):
            # persistent PSUM tiles, one per token tile; reused every ot so
            # WAR deps are per-bank (matmul waits only on its own bank's
            # eviction, not a pool-rotation barrier)
            ps = [pspool.tile([P, O_TILE], dt.float32, tag=f"ps{t}", name=f"ps{t}")
                  for t in range(N_TT)]

            # warmup: PE ramps to full p-state during the prologue DMAs
            z = xpool.tile([P, 2, P], dt.float8e4, tag="z", name="z")
            nc.vector.memset(z[:], 0)
            for i in range(N_WARM):
                nc.tensor.matmul(ps[0][:, :P], z[:], z[:],
                                 start=True, stop=True, perf_mode=DR)

            xq_t = [xpool.tile([P, 2, T], dt.float8e4, tag=f"xq{k}", name=f"xq{k}")
                    for k in range(KG)]
            xr_t = [xpool.tile([P, 2, T], dt.float8e4, tag=f"xr{k}", name=f"xr{k}")
                    for k in range(KG)]

            def alloc_w(ot):
                wq = [wpool.tile([P, WCH, 2, O_TILE], dt.float8e4, tag=f"wq{c}",
                                 name=f"wq{ot}_{c}") for c in range(NCH)]
                wr = [wpool.tile([P, WCH, 2, O_TILE], dt.float8e4, tag=f"wr{c}",
                                 name=f"wr{ot}_{c}") for c in range(NCH)]
                return wq, wr

            def dma_w_chunk(ws, w_d, ot, c):
                nc.sync.dma_start(ws[c][:], w_d[ot, :, WCH * c:WCH * (c + 1), :, :])

            def w_sl(ws, kg):
                return ws[kg // WCH][:, kg % WCH, :, :]

            # prologue DMA: interleaved in exactly the order ot=0 consumes
            w0q, w0r = alloc_w(0)
            for c in range(NCH):
                dma_w_chunk(w0q, wq_d, 0, c)
                nc.sync.dma_start(xq_t[2 * c][:], xq_d[:, 2 * c, :, :])
                dma_w_chunk(w0r, wr_d, 0, c)
                nc.sync.dma_start(xr_t[2 * c][:], xr_d[:, 2 * c, :, :])
                nc.sync.dma_start(xq_t[2 * c + 1][:], xq_d[:, 2 * c + 1, :, :])
                nc.sync.dma_start(xr_t[2 * c + 1][:], xr_d[:, 2 * c + 1, :, :])

            # W[1] prefetch issues right behind the prologue (ahead of ot0's
            # eviction stores in the in-order DMA queue)
            w1q, w1r = alloc_w(1)
            for c in range(NCH):
                dma_w_chunk(w1q, wq_d, 1, c)
                dma_w_chunk(w1r, wr_d, 1, c)

            def evict(tt, ot, last=False):
                st = opool.tile([P, O_TILE], dt.float32)
                h = 3 * O_TILE // 4 if last else O_TILE // 2
                nc.vector.tensor_copy(out=st[:, :h], in_=ps[tt][:, :h])
                nc.sync.dma_start(out_d[ot, tt, :, 0:h], st[:, :h])
                nc.scalar.copy(st[:, h:], ps[tt][:, h:])
                nc.sync.dma_start(out_d[ot, tt, :, h:O_TILE], st[:, h:])

            PASSES = ((xq_t, "q"), (xq_t, "r"), (xr_t, "q"))

            def mm(tt, kg, xp, wsel, wq, wr, start, stop):
                nc.tensor.matmul(
                    ps[tt][:], xp[kg][:, :, tt * P:(tt + 1) * P],
                    w_sl(wq if wsel == "q" else wr, kg),
                    start=start, stop=stop, perf_mode=DR,
                )

            # ---- ot = 0: kg-streamed, all 3 passes per kg; last KG_TAIL
            # kgs go tt-outer so the 8 evictions spread out
            for k in range(KG - KG_TAIL):
                for pi, (xp, wsel) in enumerate(PASSES):
                    for tt in range(N_TT):
                        mm(tt, k, xp, wsel, w0q, w0r,
                           start=(pi == 0 and k == 0), stop=False)
            for tt in range(N_TT):
                for k in range(KG - KG_TAIL, KG):
                    for pi, (xp, wsel) in enumerate(PASSES):
                        mm(tt, k, xp, wsel, w0q, w0r, start=False,
                           stop=(pi == len(PASSES) - 1 and k == KG - 1))
                evict(tt, 0)

            # ---- ot = 1..7: weights prefetched an ot ahead; (q,q)+(r,q)
            # kg-outer, final (q,r) pass tt-outer with spread evictions
            wq_c, wr_c = w1q, w1r
            for ot in range(1, N_OT):
                wq, wr = wq_c, wr_c
                if ot + 1 < N_OT:
                    wq_c, wr_c = alloc_w(ot + 1)
                    for c in range(NCH):
                        dma_w_chunk(wq_c, wq_d, ot + 1, c)
                        dma_w_chunk(wr_c, wr_d, ot + 1, c)
                for k in range(KG):
                    for xp, wsel, first in ((xq_t, "q", True), (xq_t, "r", False)):
                        for tt in range(N_TT):
                            mm(tt, k, xp, wsel, wq, wr,
                               start=(first and k == 0), stop=False)
                for tt in range(N_TT):
                    for k in range(KG):
                        mm(tt, k, xr_t, "q", wq, wr, start=False,
                           stop=(k == KG - 1))
                    evict(tt, ot, last=(ot == N_OT - 1 and tt == N_TT - 1))

    nc.compile()
    return nc


def _split_planes(v):
    hi = v.astype(E4)
    lo = (v - hi.astype(np.float32)).astype(E4)
    return hi, lo


def _prep_inputs(x, qweight, scale, lora_A, lora_B):
    # effective dense weight with the LoRA update folded in
    w = qweight.astype(np.float32) * scale.astype(np.float32)
    w += SCALING * (lora_B.astype(np.float32) @ lora_A.astype(np.float32))
    wq, wr = _split_planes(w)

    def w_layout(p):
        # [DOUT, DIN] -> K-major rhs layout [N_OT, P, KG, 2, O_TILE],
        # K = kg*256 + sub*128 + p
        t = p.T.reshape(KG, 2, P, N_OT, O_TILE)
        return np.ascontiguousarray(t.transpose(3, 2, 0, 1, 4))

    xf = np.ascontiguousarray(x.reshape(TOK, DIN))
    xhi, xlo = _split_planes(xf)

    def x_layout(p, c):
        # core slice [T, DIN] -> lhsT layout [P, KG, 2, T]
        t = p[c * T:(c + 1) * T].T.reshape(KG, 2, P, T)
        return np.ascontiguousarray(t.transpose(2, 0, 1, 3))

    wq_l, wr_l = w_layout(wq), w_layout(wr)
    per_core = [
        {"xq": x_layout(xhi, c), "xr": x_layout(xlo, c), "wq": wq_l, "wr": wr_l}
        for c in range(N_CORES)
    ]
    return per_core


def run(x, qweight, scale, lora_A, lora_B, trace=False):
    from concourse.bass_utils import run_bass_kernel_spmd

    if "nc" not in _CACHE:
        _CACHE["nc"] = build_nc()
    nc = _CACHE["nc"]

    in_maps = _prep_inputs(x, qweight, scale, lora_A, lora_B)
    res = run_bass_kernel_spmd(nc, in_maps, core_ids=list(range(N_CORES)),
                               trace=trace)
    outs = []
    for c in range(N_CORES):
        o = res.results[c]["out"]  # [N_OT, N_TT, P, O_TILE]
        outs.append(o.transpose(1, 2, 0, 3).reshape(T, DOUT))
    full = np.concatenate(outs, axis=0).reshape(B, S, DOUT).astype(np.float32)
    return full, res


def kernel(x, qweight, scale, lora_A, lora_B):
    full, _ = run(x, qweight, scale, lora_A, lora_B)
    return full
